# revision 20
# baseline (speedup 1.0000x reference)
"""Binary linear layer (sign(X) @ sign(W) * scale) on 8 trn2 NeuronCores.

Strategy: data-parallel over the batch dim (1/8 of X's rows per core), with
the matmul TRANSPOSED vs the usual layout: W is the stationary operand and X
streams, so PSUM tiles are [128 n-cols, 512 m-rows]. The scale vector is then
PER-PARTITION, which lets the Scalar (ACT) engine do the whole eviction as
one activation (out = psum * scale[:,1]) -- no [128, N] scale broadcast, no
DVE dependency, and the host precomputes relu(alpha)*outer(relu(betta),
relu(gamma)) as a tiny [128, 16] table. Y is produced n-major per core and
transposed on the host.

Why transposed: each PSUM accumulation group (nt, m-half) needs only a
[128, 2, 128] W slice per k-chunk, so the DMA-paced opening phase needs
XP_k (256KB) + a QUARTER of W_k (128KB) = 225 GB/s, under the ~285 GB/s one
HWDGE queue delivers -- the old X-stationary layout needed 300 GB/s and
stalled. All inputs ride the sync queue in exact consumption order.

Schedule (from perfetto traces): steady-state [128,2,512] fp8 DoubleRow
matmul streams 512 cols in ~213ns at full clock; 512 MMs/core = ~109.2us PE
floor. Phase A: k-outer over 8 PSUM banks (nt 0-3 x mhalf 0-1) paced by the
input DMA. Phase B: one (nt, mhalf) pair per bank, k-inner, banks rotating
mod 8 so each eviction has ~13us of slack and the PE never waits on a bank.
The last nt runs its two pairs serially so the tail after the final MM is a
single ACT eviction + one 256KB store. Junk MMs at t=0 pre-ramp the HAM
clock gate (it ramps 0.65 -> 2.4 GHz only over ~3-4us of CONTINUOUS PE busy)
while the first input chunks are in flight; no other junk is needed because
the bank rotation leaves no group boundaries.
"""

import os

import numpy as np

import concourse.bacc as bacc
import concourse.mybir as mybir
from concourse.tile import TileContext

P = 128
B, IN, OUT = 8192, 4096, 2048
NCORES = 8
M = B // NCORES  # 1024 rows per core
K = IN
N = OUT
NPAIR = K // (2 * P)  # 16 k-pair blocks; block i covers k = [i*256, (i+1)*256)
NNT = N // P  # 16 n-tiles of 128 (PSUM partition dim)
FD = 512  # psum tile free dim (one bank) = m-half size
NMH = M // FD  # 2 m-halves
NQ = 4  # W column quarters (4 nt blocks each)


def build_kernel(
    warmup_mms=14,  # junk PE matmuls at t=0: pre-ramp the HAM clock gate
    out_bufs=8,
    ev="act",  # eviction engine: "act" (scalar) | "dve" | "alt"
    split_first=True,  # split k=0 loads so the very first MM starts earlier
    alt_store=True,  # phase-B stores alternate scalar/sync HWDGE queues
    last_split=True,  # final eviction: halves on ACT+DVE, stores on both queues
    warm_dma=False,  # tiny leading DMA (measured: hurts; keeps knob for A/B)
):
    f32 = mybir.dt.float32
    fp8 = mybir.dt.float8e4
    pm = mybir.MatmulPerfMode.DoubleRow
    AF = mybir.ActivationFunctionType

    nc = bacc.Bacc("TRN2", debug=False, num_devices=NCORES)

    # XP[i*128+p, s, m] = sign(X)[m-th row of this core's slice, k]
    # with k = 2*(i*128+p)+s (pair-packed; see _make_in_maps)
    XP = nc.declare_dram_parameter("XP", [NPAIR * P, 2, M], fp8, isOutput=False)
    # WQ[q, i*128+p, s, c] = sign(W)[k, q*512 + c], same k map; quarter q
    # covers nt blocks 4q..4q+3
    WQ = nc.declare_dram_parameter("WQ", [NQ, NPAIR * P, 2, N // NQ], fp8, isOutput=False)
    # SC[p, t] = relu(alpha) * outer(relu(betta), relu(gamma)).flat[t*128+p]
    SC = nc.declare_dram_parameter("SC", [P, NNT], f32, isOutput=False)
    Y = nc.declare_dram_parameter("Y", [N, M], f32, isOutput=True)  # n-major!

    with TileContext(nc) as tc:
        with (
            tc.tile_pool(name="const", bufs=1) as cpool,
            tc.tile_pool(name="bin", bufs=2) as binpool,
            tc.tile_pool(name="outp", bufs=2) as outpool,
            tc.tile_pool(name="psum", bufs=8, space="PSUM") as pspool,
        ):
            # ---- PE warm-up: no-dep junk matmuls bridge the gap between the
            # framework preamble and the first input chunk's arrival
            # (~10.3us; the first transfers pay the DMA engines' cold-start).
            # The memset rides gpsimd right after the framework's own
            # memsets there, so the PE goes busy ~6.7us and the HAM
            # clock-ramp gate stays fed until real data lands. ----
            wu = cpool.tile([P, 2, 256], fp8, bufs=1)
            nc.vector.memset(wu, 0)
            ps_wu = pspool.tile([P, FD], f32, tag="mm", bufs=8)
            for _ in range(warmup_mms):
                nc.tensor.matmul(
                    ps_wu[:, :256],
                    lhsT=wu[:, :, :P],
                    rhs=wu,
                    start=True,
                    stop=True,
                    perf_mode=pm,
                )

            # tiny per-partition scale table, SWDGE (off the bulk queues);
            # after the wu memset so the junk matmuls start sooner
            scale_t = cpool.tile([P, NNT], f32, bufs=1)
            nc.gpsimd.dma_start(out=scale_t, in_=SC[:, :])

            # ---- input loads, all on the sync HWDGE queue, in exact
            # consumption order: (WQ0_k, XP_k) pairs for phase A, then the
            # remaining W quarters for phase B. A 2KB dummy load goes first:
            # the first transfer on a cold queue pays ~2us of descriptor/
            # engine spin-up, and the dummy pays it instead of WQ0_0. ----
            if warm_dma:
                warm = cpool.tile([1, 2, M], fp8, bufs=1)
                nc.sync.dma_start(out=warm, in_=XP[0:1])
            xbs = []
            wqs = [[None] * NPAIR for _ in range(NQ)]
            for i in range(NPAIR):
                xb = binpool.tile([P, 2, M], fp8, tag="xb", bufs=NPAIR)
                wb = binpool.tile([P, 2, N // NQ], fp8, tag="wb", bufs=NQ * NPAIR)
                if i == 0 and split_first:
                    # first MM needs WQ0_0 + the m-half-0 part of XP_0 only
                    nc.sync.dma_start(out=wb, in_=WQ[0, :P])
                    nc.sync.dma_start(out=xb[:, :, :FD], in_=XP[:P, :, :FD])
                    nc.sync.dma_start(out=xb[:, :, FD:], in_=XP[:P, :, FD:])
                else:
                    nc.sync.dma_start(out=wb, in_=WQ[0, i * P : (i + 1) * P])
                    nc.sync.dma_start(out=xb, in_=XP[i * P : (i + 1) * P])
                xbs.append(xb)
                wqs[0][i] = wb
            for q in range(1, NQ):
                for i in range(NPAIR):
                    wb = binpool.tile([P, 2, N // NQ], fp8, tag="wb", bufs=NQ * NPAIR)
                    nc.sync.dma_start(out=wb, in_=WQ[q, i * P : (i + 1) * P])
                    wqs[q][i] = wb

            def lhsT_of(nt, i):
                q, j = divmod(nt, NQ)
                return wqs[q][i][:, :, j * P : (j + 1) * P]

            def rhs_of(i, mh):
                return xbs[i][:, :, mh * FD : (mh + 1) * FD]

            n_ev = [0]

            def evict(ps, nt, mh, alt=False):
                ot = outpool.tile([P, FD], f32, tag="ot", bufs=out_bufs)
                use_dve = ev == "dve" or (ev == "alt" and n_ev[0] % 2 == 1)
                if use_dve:
                    nc.vector.tensor_scalar_mul(ot, ps, scale_t[:, nt : nt + 1])
                else:
                    nc.scalar.activation(
                        ot, ps, AF.Copy, scale=scale_t[:, nt : nt + 1]
                    )
                # the sync HWDGE queue is done with inputs by ~42us; sharing
                # the Y stores across both queues halves the store latency
                q = nc.sync if (alt and n_ev[0] % 2 == 1) else nc.scalar
                n_ev[0] += 1
                q.dma_start(
                    out=Y[nt * P : (nt + 1) * P, mh * FD : (mh + 1) * FD],
                    in_=ot,
                )

            def evict_split(ps, nt, mh):
                # final eviction: one full ACT (PSUM reads from different
                # engines serialize anyway), then the two halves store on
                # BOTH HWDGE queues in parallel, halving the store tail
                h = FD // 2
                o = outpool.tile([P, FD], f32, tag="ot", bufs=out_bufs)
                nc.scalar.activation(o, ps, AF.Copy, scale=scale_t[:, nt : nt + 1])
                base = mh * FD
                nc.scalar.dma_start(
                    out=Y[nt * P : (nt + 1) * P, base : base + h], in_=o[:, :h]
                )
                nc.sync.dma_start(
                    out=Y[nt * P : (nt + 1) * P, base + h : base + FD], in_=o[:, h:]
                )

            # ---- phase A: k-outer over 8 banks (nt 0-3 x mhalf 0-1), paced
            # by the input DMA; W is stationary so each k needs only 384KB ----
            banksA = {}
            for nt in range(4):
                for mh in range(NMH):
                    banksA[(nt, mh)] = pspool.tile(
                        [P, FD], f32, tag="mm", bufs=8, name=f"psA_{nt}_{mh}"
                    )
            for i in range(NPAIR):
                for nt in range(4):
                    lhsT = lhsT_of(nt, i)
                    for mh in range(NMH):
                        nc.tensor.matmul(
                            banksA[(nt, mh)],
                            lhsT=lhsT,
                            rhs=rhs_of(i, mh),
                            start=(i == 0),
                            stop=(i == NPAIR - 1),
                            perf_mode=pm,
                        )
            for nt in range(4):
                for mh in range(NMH):
                    evict(banksA[(nt, mh)], nt, mh)

            # ---- phase B: nt 4..15, k-inner, banks rotating mod 8; the two
            # m-halves of one nt interleave so each weight slice is used by
            # both back-to-back. The final nt runs its halves serially so the
            # tail after the last MM is one eviction, not two. ----
            for nt in range(4, NNT - 1):
                bs = [
                    pspool.tile([P, FD], f32, tag="mm", bufs=8, name=f"psB_{nt}_{mh}")
                    for mh in range(NMH)
                ]
                for i in range(NPAIR):
                    lhsT = lhsT_of(nt, i)
                    for mh in range(NMH):
                        nc.tensor.matmul(
                            bs[mh],
                            lhsT=lhsT,
                            rhs=rhs_of(i, mh),
                            start=(i == 0),
                            stop=(i == NPAIR - 1),
                            perf_mode=pm,
                        )
                for mh in range(NMH):
                    evict(bs[mh], nt, mh, alt=alt_store)
            nt = NNT - 1
            for mh in range(NMH):
                b = pspool.tile([P, FD], f32, tag="mm", bufs=8, name=f"psL_{mh}")
                for i in range(NPAIR):
                    nc.tensor.matmul(
                        b,
                        lhsT=lhsT_of(nt, i),
                        rhs=rhs_of(i, mh),
                        start=(i == 0),
                        stop=(i == NPAIR - 1),
                        perf_mode=pm,
                    )
                if last_split and mh == NMH - 1:
                    evict_split(b, nt, mh)
                else:
                    evict(b, nt, mh, alt=alt_store)
    return nc


_NC_CACHE = {}


def _get_nc(**kw):
    key = tuple(sorted(kw.items()))
    if key not in _NC_CACHE:
        nc = build_kernel(**kw)
        nc.finalize()
        _NC_CACHE[key] = nc
    return _NC_CACHE[key]


def _make_in_maps(X, W, alpha, betta, gamma):
    fp8 = mybir.dt.np(mybir.dt.float8e4)
    X = np.asarray(X, dtype=np.float32)
    W = np.asarray(W, dtype=np.float32)
    # +-1 is exact in fp8e4m3, so the device matmul is bit-identical to
    # sign(X) @ sign(W)
    Wb = np.sign(W).astype(fp8)  # [K, N]
    # WQ[q, r, s, c] = Wb[2r+s, q*512+c]: pair-packed 1024B rows, quartered
    # along N so phase A only pulls the first quarter
    WQv = np.ascontiguousarray(
        Wb.reshape(K // 2, 2, NQ, N // NQ).transpose(2, 0, 1, 3)
    )
    # host-side scale table: relu is free here and the module applies it
    # before the functional call anyway
    a = float(np.maximum(np.asarray(alpha, dtype=np.float32), 0.0))
    bv = np.maximum(np.asarray(betta, dtype=np.float32).reshape(32), 0.0)
    gv = np.maximum(np.asarray(gamma, dtype=np.float32).reshape(64), 0.0)
    scale = (a * np.outer(bv, gv)).reshape(-1).astype(np.float32)  # [N]
    SCt = np.ascontiguousarray(scale.reshape(NNT, P).T)  # [128, 16]
    in_maps = []
    for c in range(NCORES):
        xs = np.sign(X[c * M : (c + 1) * M, :]).astype(fp8)  # [M, K]
        # XP[r, s*1024+m] = sign(X).T[2r+s, m] -- a contiguous k-major
        # transpose pair-packs rows for free
        xp = np.ascontiguousarray(xs.T).reshape(K // 2, 2, M)
        in_maps.append({"XP": xp, "WQ": WQv, "SC": SCt})
    return in_maps


def run_on_cores(inputs, trace=False, tmpdir=None, **build_kw):
    """Run the SPMD kernel on 8 cores; returns (Y_full, BassKernelResults)."""
    from concourse.bass_utils import run_bass_kernel_spmd

    if not trace:
        # this image lacks antenv.axon_hooks; a stray BASS_TRACE env var would
        # crash run_bass_kernel_spmd's trace branch, so fail safe
        try:
            import antenv.axon_hooks  # noqa: F401
        except ImportError:
            os.environ.setdefault("BASS_NEVER_TRACE", "1")
    nc = _get_nc(**build_kw)
    in_maps = _make_in_maps(**inputs)
    res = run_bass_kernel_spmd(
        nc, in_maps, list(range(NCORES)), trace=trace, tmpdir=tmpdir
    )
    # device Y is [N, M] per core (n-major); transpose + stack on the host
    Yf = np.concatenate([np.ascontiguousarray(r["Y"].T) for r in res.results], axis=0)
    return Yf, res


PROD_KW = dict(
    warmup_mms=14,
    out_bufs=8,
    ev="act",
    split_first=True,
    alt_store=True,
    last_split=True,
    warm_dma=False,
)


def kernel(**inputs) -> np.ndarray:
    Yr, _ = run_on_cores(inputs, **PROD_KW)
    return Yr
